# revision 47
# baseline (speedup 1.0000x reference)
"""Fused attention block (qkv proj + pooled attention + 16-head masked
attention + out proj) for TRN2, batch-parallel across 8 NeuronCores.

Structural optimizations vs a direct port:
  * Key/query compaction: the mask is known host-side; masked keys
    contribute exactly zero after softmax and masked-query rows get
    exactly uniform attention (one host-computed row ybar).  The headed
    path runs on a compacted sequence padded to KP (chosen per call
    from max n_keep, 64-aligned; n_keep ~ 1+Binom(1023,.5)).
  * Pooled attention via G-trick: attn_ = softmax(x (Wq Wk^T) x^T / 32)
    with G precomputed on host, saving one full projection.
  * AV runs in [query, feature] orientation (P^T tiles as stationary)
    which fills all 128 output partitions: half the PE rows of the
    feature-major form, and row-sum normalization becomes a plain
    per-partition tensor_scalar_mul (sums land as extra PSUM columns).
  * Head pairs are interleaved with the projection/pooled matmul work
    so the ACT-engine exp stream hides under PE work.
  * bf16 operands on the PE (cost-model rate matches fp32r, halves
    DMA/SBUF); fp32 PSUM accumulation; masked-row fill, +b_out, and
    scatter to full [n, d] happen on host.

Outputs: attn [n, n] fp32 (full pooled), outc [KP, D] fp32 (compact).
"""
import os
import sys
from contextlib import ExitStack

sys.path.insert(0, "/opt/trn_rl_repo")

import numpy as np
import ml_dtypes

import concourse.bass as bass
import concourse.mybir as mybir
import concourse.tile as tile
from concourse import bacc, bass_utils

F32 = mybir.dt.float32
BF16 = mybir.dt.bfloat16
EXP = mybir.ActivationFunctionType.Exp
BF = ml_dtypes.bfloat16

B = 8
N = 1024          # sequence (after CLS pad)
D = 1024          # model dim
H = 16
DH = 64
NT = N // 128     # 8 row tiles
KP_MAX = 896      # av_t [128, KP+2*KT] fp32 must fit 2 PSUM banks
SCALE_H = DH ** -0.5     # 1/8
SCALE_P = D ** -0.5      # 1/32
NEG = -80.0

_CACHED = {}


def _tiles(kp):
    """Widths of the 128-partition tiles covering kp (last may be <128)."""
    ws = [128] * (kp // 128)
    if kp % 128:
        ws.append(kp % 128)
    return ws


def build_nc(kp):
    tw_list = _tiles(kp)
    KT = len(tw_list)
    CS = [(0, min(512, kp))] + ([(512, kp - 512)] if kp > 512 else [])
    scol = 128 * len(tw_list)  # AV blocks form a (it, u) grid of 64-wide cols

    nc = bacc.Bacc("TRN2", target_bir_lowering=False, debug=False, num_devices=8)
    xt_d = nc.dram_tensor("xt", [D, N], BF16, kind="ExternalInput").ap()
    xct_d = nc.dram_tensor("xct", [D, kp], BF16, kind="ExternalInput").ap()
    g_d = nc.dram_tensor("g", [D, D], BF16, kind="ExternalInput").ap()
    wq_d = nc.dram_tensor("wq", [D, D], BF16, kind="ExternalInput").ap()
    wk_d = nc.dram_tensor("wk", [D, D], BF16, kind="ExternalInput").ap()
    wv_d = nc.dram_tensor("wv", [D, D], BF16, kind="ExternalInput").ap()
    wo_d = nc.dram_tensor("wo", [D, D], BF16, kind="ExternalInput").ap()
    cbias_d = nc.dram_tensor("cbias", [128, KT], F32, kind="ExternalInput").ap()
    attn_d = nc.dram_tensor("attn", [N, N], F32, kind="ExternalOutput").ap()
    outc_d = nc.dram_tensor("outc", [kp, D], F32, kind="ExternalOutput").ap()

    def ceng():
        # GPSIMD cannot access PSUM; every element-wise op here reads
        # PSUM, so they all go to the DVE.
        return nc.vector

    with tile.TileContext(nc, trace_sim=bool(os.environ.get('ATTN_TRACE_SIM'))) as tc:
        with ExitStack() as st:
            def P(name, bufs, **kw):
                return st.enter_context(tc.tile_pool(name=name, bufs=bufs, **kw))
            xtp = P("xtp", 8)
            xctp = P("xctp", 8)
            gp = P("gp", 8)
            wqp = P("wqp", 8)
            wkp = P("wkp", 8)
            wvp = P("wvp", 8)
            wop = P("wop", 8)
            atp = P("atp", 8)
            qcp = P("qcp", 8)
            kcp = P("kcp", 8)
            vcp = P("vcp", KT)
            aoifp = P("aoifp", KT)
            aotp = P("aotp", 8)
            ptp = P("ptp", 5)
            pep = P("pep", 4)
            otp = P("otp", 5)
            onep = P("one", 1)
            smallp = P("small", 6)
            dpp = P("dp", 2, space="PSUM")    # [128, kp] f32: dots tiles
            avp = P("av", 1, space="PSUM")    # [128, scol+2*KT] f32
            fap = P("fa", 2, space="PSUM")    # [128, 512] f32 filler halves

            # ---- constants / bulk loads ----
            ones_t = onep.tile([128, 1], BF16, name="ones_t", tag="ones")
            nc.vector.memset(ones_t, 1.0)
            xct_t = []
            for t in range(NT):
                x_ = xctp.tile([128, kp], BF16, tag="xct", name=f"xct{t}")
                if t == 0:
                    # split first transfer: the first vc matmuls need only
                    # cols 0:384 (token tiles 0-2), so PE starts earlier
                    nc.gpsimd.dma_start(out=x_[:, 0:384],
                                        in_=xct_d[0:128, 0:384])
                    nc.gpsimd.dma_start(out=x_[:, 384:kp],
                                        in_=xct_d[0:128, 384:kp])
                else:
                    nc.gpsimd.dma_start(out=x_,
                                        in_=xct_d[t * 128:(t + 1) * 128, :])
                xct_t.append(x_)
            cbias_t = onep.tile([128, KT], F32, name="cbias_t", tag="cbias")
            nc.gpsimd.dma_start(out=cbias_t, in_=cbias_d)
            wv_t = []
            for t in range(NT):
                w = wvp.tile([128, D], BF16, tag="wv", name=f"wv{t}")
                if t == 0:
                    # halved first transfer: the first vc matmul only needs
                    # cols 0:512, so PE can start half a transfer earlier
                    nc.sync.dma_start(out=w[:, 0:512], in_=wv_d[0:128, 0:512])
                    nc.sync.dma_start(out=w[:, 512:D], in_=wv_d[0:128, 512:D])
                else:
                    nc.sync.dma_start(out=w,
                                      in_=wv_d[t * 128:(t + 1) * 128, :])
                wv_t.append(w)
            wq_t = []
            for t in range(NT):
                w = wqp.tile([128, D], BF16, tag="wq", name=f"wq{t}")
                nc.sync.dma_start(out=w, in_=wq_d[t * 128:(t + 1) * 128, :])
                wq_t.append(w)
            wk_t = []
            for t in range(NT):
                w = wkp.tile([128, D], BF16, tag="wk", name=f"wk{t}")
                nc.gpsimd.dma_start(out=w, in_=wk_d[t * 128:(t + 1) * 128, :])
                wk_t.append(w)
            g_t = []
            for t in range(NT):
                g_ = gp.tile([128, D], BF16, tag="g", name=f"g{t}")
                nc.scalar.dma_start(out=g_, in_=g_d[t * 128:(t + 1) * 128, :])
                g_t.append(g_)
            xt_t = []
            for t in range(NT):
                x_ = xtp.tile([128, N], BF16, tag="xt", name=f"xt{t}")
                nc.scalar.dma_start(out=x_, in_=xt_d[t * 128:(t + 1) * 128, :])
                xt_t.append(x_)
            wo_t = []
            for t in range(NT):
                w = wop.tile([128, D], BF16, tag="wo", name=f"wo{t}")
                nc.gpsimd.dma_start(out=w, in_=wo_d[t * 128:(t + 1) * 128, :])
                wo_t.append(w)

            # ---- vc projection (position-major [token, feature]) ----
            # The first 6 us are paced by the wv DMA stream; open five
            # accumulation series at once (fa x2, dp x2, av x1 pools) so each
            # arriving wv tile feeds five matmuls instead of one.
            vc_t = [vcp.tile([128, D], BF16, tag="vc", name=f"vc{t}")
                    for t in range(KT)]
            early = [(0, 0), (0, 1), (1, 0), (1, 1), (2, 0)]
            pss = {}
            pss[(0, 0)] = fap.tile([128, 512], F32, tag="fa", name="psv00")
            pss[(0, 1)] = fap.tile([128, 512], F32, tag="fa", name="psv01")
            pss[(1, 0)] = dpp.tile([128, kp], F32, tag="dp", name="psv10")
            pss[(1, 1)] = dpp.tile([128, kp], F32, tag="dp", name="psv11")
            pss[(2, 0)] = avp.tile([128, scol + 2 * KT], F32, tag="av",
                                   name="psv20")
            for d in range(NT):
                for t, c in early:
                    nc.tensor.matmul(
                        pss[(t, c)][0:tw_list[t], 0:512],
                        xct_t[d][:, t * 128:t * 128 + tw_list[t]],
                        wv_t[d][:, c * 512:(c + 1) * 512],
                        start=(d == 0), stop=(d == NT - 1))
            for t, c in early:
                ceng().tensor_copy(
                    vc_t[t][0:tw_list[t], c * 512:(c + 1) * 512],
                    pss[(t, c)][0:tw_list[t], 0:512])
            ndp = 0
            for t in range(KT):
                tw = tw_list[t]
                for c in range(2):
                    if (t, c) in pss:
                        continue
                    if ndp < 2:
                        # dp slots stay idle until the first head pair;
                        # borrowing them widens the vc->heads transition
                        ps = dpp.tile([128, kp], F32, tag="dp",
                                      name=f"psv{t}{c}")
                        ndp += 1
                    else:
                        ps = fap.tile([128, 512], F32, tag="fa",
                                      name=f"psv{t}{c}")
                    for d in range(NT):
                        nc.tensor.matmul(
                            ps[0:tw, 0:512],
                            xct_t[d][:, t * 128:t * 128 + tw],
                            wv_t[d][:, c * 512:(c + 1) * 512],
                            start=(d == 0), stop=(d == NT - 1))
                    ceng().tensor_copy(vc_t[t][0:tw, c * 512:(c + 1) * 512],
                                       ps[0:tw, 0:512])

            # ---- qc/kc projection (feature-major compact) ----
            qc_t = [None] * NT
            kc_t = [None] * NT

            def qk_proj(f, use_dp=False):
                qc = qcp.tile([128, kp], BF16, tag="qc", name=f"qc{f}")
                kc = kcp.tile([128, kp], BF16, tag="kc", name=f"kc{f}")
                rem = kp - 512
                main_cs = [(0, min(512, kp))] if 0 < rem <= 256 else CS
                for w_t, dst, tag in ((wq_t, qc, "qc"), (wk_t, kc, "kc")):
                    for c0, cw in main_cs:
                        if use_dp:
                            ps = dpp.tile([128, kp], F32, tag="dp",
                                          name=f"ps{tag}{f}{c0}")
                        else:
                            ps = fap.tile([128, 512], F32, tag="fa",
                                          name=f"ps{tag}{f}{c0}")
                        for d in range(NT):
                            nc.tensor.matmul(
                                ps[:, 0:cw],
                                w_t[d][:, f * 128:(f + 1) * 128],
                                xct_t[d][:, c0:c0 + cw],
                                start=(d == 0), stop=(d == NT - 1))
                        ceng().tensor_copy(dst[:, c0:c0 + cw], ps[:, 0:cw])
                if 0 < rem <= 256:
                    # tails share one PSUM slot (bank): the first series'
                    # start=True pending-zeroes the bank, the second series'
                    # first write lands fresh on pending bytes
                    ps1 = fap.tile([128, 512], F32, tag="fa", name=f"psc1{f}")
                    for i, (w_t, dst) in enumerate(((wq_t, qc), (wk_t, kc))):
                        for d in range(NT):
                            nc.tensor.matmul(
                                ps1[:, i * rem:(i + 1) * rem],
                                w_t[d][:, f * 128:(f + 1) * 128],
                                xct_t[d][:, 512:kp],
                                start=(d == 0 and i == 0),
                                stop=(d == NT - 1), skip_group_check=True)
                    for i, dst in enumerate((qc, kc)):
                        ceng().tensor_copy(dst[:, 512:kp],
                                           ps1[:, i * rem:(i + 1) * rem])
                qc_t[f] = qc
                kc_t[f] = kc

            def at_tile(dt):
                at = atp.tile([128, N], BF16, tag="at", name=f"at{dt}")
                for c in range(2):
                    ps = fap.tile([128, 512], F32, tag="fa", name=f"psat{dt}{c}")
                    for dk in range(NT):
                        nc.tensor.matmul(
                            ps,
                            g_t[dk][:, dt * 128:(dt + 1) * 128],
                            xt_t[dk][:, c * 512:(c + 1) * 512],
                            start=(dk == 0), stop=(dk == NT - 1))
                    ceng().tensor_copy(at[:, c * 512:(c + 1) * 512], ps)
                return at

            qk_proj(0, use_dp=True)

            # ---- heads interleaved with remaining projections ----
            ao_if = []
            for t in range(KT):
                ao = aoifp.tile([128, D], BF16, tag="aoif", name=f"aoif{t}")
                ao_if.append(ao)
            at_t = [None] * NT

            for hp in range(NT):
                # emit next head-pair's projections first so their PSUM
                # copies have a full head-pair of slack before they're read
                if hp + 1 < NT:
                    qk_proj(hp + 1)
                av_t = avp.tile([128, scol + 2 * KT], F32, tag="av",
                                name=f"av{hp}")
                # start=True zeroes the whole 2KB PSUM bank (pending-zero),
                # so with many accumulation series per bank only the FIRST
                # emitted write of each bank may carry start=True; the other
                # series' first writes land on pending-zero bytes and
                # overwrite-fresh.
                seen_banks = set()

                def av_start(jt, byte_off):
                    if jt != 0:
                        return False
                    bank = byte_off // 2048
                    if bank in seen_banks:
                        return False
                    seen_banks.add(bank)
                    return True

                for jt in range(KT):
                    j0, jw = jt * 128, tw_list[jt]
                    for u in range(2):
                        off = u * 64
                        dp = dpp.tile([128, kp], F32, tag="dp",
                                      name=f"dp{hp}{u}{jt}")
                        ks = kc_t[hp][off:off + 64, j0:j0 + jw]
                        for c0, cw in CS:
                            nc.tensor.matmul(
                                dp[0:jw, c0:c0 + cw], ks,
                                qc_t[hp][off:off + 64, c0:c0 + cw],
                                start=True, stop=True)
                        pt = ptp.tile([128, kp], BF16, tag="pt",
                                      name=f"pt{hp}{u}{jt}")
                        nc.scalar.activation(
                            pt[0:jw, :], dp[0:jw, :], EXP,
                            bias=cbias_t[0:jw, jt:jt + 1], scale=SCALE_H)
                        h = 2 * hp + u
                        for it in range(KT):
                            i0, iw = it * 128, tw_list[it]
                            stat = pt[0:jw, i0:i0 + iw]
                            nc.tensor.matmul(
                                av_t[0:iw, i0 + off:i0 + off + 64],
                                stat, vc_t[jt][0:jw, h * 64:(h + 1) * 64],
                                start=av_start(jt, (i0 + off) * 4),
                                stop=(jt == KT - 1), skip_group_check=True)
                            nc.tensor.matmul(
                                av_t[0:iw, scol + it * 2 + u:
                                     scol + it * 2 + u + 1],
                                stat, ones_t[0:jw, :],
                                start=av_start(jt, (scol + it * 2 + u) * 4),
                                stop=(jt == KT - 1), skip_group_check=True)
                # normalization: per-partition scales from the sums columns
                rec = smallp.tile([128, 2 * KT], F32, tag="rec",
                                  name=f"rec{hp}")
                nfull = 2 * (KT - 1) if tw_list[-1] < 128 else 2 * KT
                nc.vector.reciprocal(rec[:, 0:nfull],
                                     av_t[:, scol:scol + nfull])
                if nfull < 2 * KT:
                    lw = tw_list[-1]
                    nc.vector.reciprocal(
                        rec[0:lw, nfull:2 * KT],
                        av_t[0:lw, scol + nfull:scol + 2 * KT])
                for it in range(KT):
                    i0, iw = it * 128, tw_list[it]
                    for u in range(2):
                        h = 2 * hp + u
                        ceng().tensor_scalar_mul(
                            ao_if[it][0:iw, h * 64:(h + 1) * 64],
                            av_t[0:iw, i0 + u * 64:i0 + u * 64 + 64],
                            rec[0:iw, it * 2 + u:it * 2 + u + 1])

                at_t[hp] = at_tile(hp)


            # ---- tail: transpose ao, out-projection, pooled attention ----
            aoT_t = []
            for f in range(NT):
                aoT = aotp.tile([128, kp], BF16, tag="aot", name=f"aot{f}")
                aoT_t.append(aoT)

            def pooled_tile(it):
                sums = []
                pes = []
                pss = []
                for c in range(2):
                    ps = fap.tile([128, 512], F32, tag="fa", name=f"psp{it}{c}")
                    for dt in range(NT):
                        nc.tensor.matmul(
                            ps,
                            at_t[dt][:, it * 128:(it + 1) * 128],
                            xt_t[dt][:, c * 512:(c + 1) * 512],
                            start=(dt == 0), stop=(dt == NT - 1))
                    pe = pep.tile([128, 512], BF16, tag="pe",
                                  name=f"pe{it}{c}")
                    sm = smallp.tile([128, 1], F32, tag=f"sm{c}",
                                     name=f"sm{it}{c}")
                    nc.scalar.activation(pe, ps, EXP, scale=SCALE_P,
                                         accum_out=sm)
                    sums.append(sm)
                    pes.append(pe)
                    pss.append(ps)
                st_ = smallp.tile([128, 1], F32, tag="st", name=f"st{it}")
                nc.vector.tensor_add(st_, sums[0], sums[1])
                rc = smallp.tile([128, 1], F32, tag="rc", name=f"rc{it}")
                nc.vector.reciprocal(rc, st_)
                for c in range(2):
                    ot = otp.tile([128, 512], F32, tag="ot", name=f"ota{it}{c}")
                    ceng().tensor_scalar_mul(ot, pes[c], rc)
                    q = nc.sync if c == 0 else nc.gpsimd
                    q.dma_start(
                        out=attn_d[it * 128:(it + 1) * 128,
                                   c * 512:(c + 1) * 512], in_=ot)

            # pooled tiles fill the PE while transposes/out-proj chase their
            # dependencies; the final emitted work is out-proj (short tail).
            pooled_plan = {t: [] for t in range(KT)}
            for i in range(NT):
                pooled_plan[min(i // 2, KT - 1)].append(i)
            for t in range(KT):
                tw = tw_list[t]
                for f in range(NT):
                    # XBAR DMA transpose: [tw, 128] block -> feature-major
                    q = nc.sync if f % 2 else nc.scalar
                    q.dma_start_transpose(
                        aoT_t[f][:, t * 128:t * 128 + tw],
                        ao_if[t][0:tw, f * 128:(f + 1) * 128])
                for it in pooled_plan[t]:
                    pooled_tile(it)
                # out projection for this token tile
                for c in range(2):
                    ps = fap.tile([128, 512], F32, tag="fa", name=f"pso{t}{c}")
                    for f in range(NT):
                        nc.tensor.matmul(
                            ps[0:tw, :],
                            aoT_t[f][:, t * 128:t * 128 + tw],
                            wo_t[f][:, c * 512:(c + 1) * 512],
                            start=(f == 0), stop=(f == NT - 1))
                    ot = otp.tile([128, 512], F32, tag="ot", name=f"oto{t}{c}")
                    # the very last chunk drains in two pieces to shorten
                    # the copy->DMA tail chain
                    pieces = ((0, 256), (256, 256)) if (
                        t == KT - 1 and c == 1) else ((0, 512),)
                    for p0, pw in pieces:
                        ceng().tensor_copy(ot[0:tw, p0:p0 + pw],
                                           ps[0:tw, p0:p0 + pw])
                        nc.sync.dma_start(
                            out=outc_d[t * 128:t * 128 + tw,
                                       c * 512 + p0:c * 512 + p0 + pw],
                            in_=ot[0:tw, p0:p0 + pw])

    nc.compile()
    return nc


def _host_prep(x, mask, w_qkv, w_out, b_out, kp):
    wq = np.ascontiguousarray(w_qkv[:, 0:D])
    wk = np.ascontiguousarray(w_qkv[:, D:2 * D])
    wv = np.ascontiguousarray(w_qkv[:, 2 * D:])
    G16 = (wq @ wk.T).astype(BF)
    wq16 = wq.astype(BF)
    wk16 = wk.astype(BF)
    wv16 = wv.astype(BF)
    wo16 = w_out.astype(BF)
    KT = len(_tiles(kp))
    in_maps, metas = [], []
    for b in range(B):
        m = np.concatenate([[True], mask[b]]).astype(bool)
        keep = np.flatnonzero(m)
        nk = len(keep)
        xb = x[b]
        xcT = np.zeros((D, kp), BF)
        if nk <= kp:
            xcT[:, :nk] = xb[keep].T
        cb = np.full(KT * 128, NEG, np.float32)
        cb[:min(nk, kp)] = 0.0
        in_maps.append({
            "xt": np.ascontiguousarray(xb.T).astype(BF),
            "xct": xcT,
            "g": G16,
            "wq": wq16,
            "wk": wk16,
            "wv": wv16,
            "wo": wo16,
            "cbias": np.ascontiguousarray(cb.reshape(KT, 128).T),
        })
        metas.append((m, keep, nk))
    return in_maps, metas, (wq, wk, wv)


def _host_heads_fallback(xb, m, wq, wk, wv, w_out, b_out):
    """Exact numpy fallback for one batch element (n_keep > kp; ~never)."""
    keep = np.flatnonzero(m)
    xc = xb[keep]
    qc, kc, vc = xc @ wq, xc @ wk, xc @ wv
    o = np.zeros((len(keep), D), np.float32)
    for h in range(H):
        sl = slice(h * DH, (h + 1) * DH)
        dd = (qc[:, sl] @ kc[:, sl].T) * SCALE_H
        P = np.exp(dd - dd.max(-1, keepdims=True))
        o[:, sl] = (P @ vc[:, sl]) / P.sum(-1, keepdims=True)
    return o @ w_out + b_out


def kernel(x, mask, w_qkv, w_out, b_out, **run_kw):
    x = np.asarray(x, np.float32)
    mask = np.asarray(mask)
    w_qkv = np.asarray(w_qkv, np.float32)
    w_out = np.asarray(w_out, np.float32)
    b_out = np.asarray(b_out, np.float32)
    max_nk = int(max(1 + mask[b].sum() for b in range(mask.shape[0])))
    kp = min(KP_MAX, max(512, ((max_nk + 31) // 32) * 32))
    if kp not in _CACHED:
        _CACHED[kp] = build_nc(kp)
    nc = _CACHED[kp]
    in_maps, metas, (wq, wk, wv) = _host_prep(x, mask, w_qkv, w_out, b_out, kp)
    try:
        res = bass_utils.run_bass_kernel_spmd(
            nc, in_maps, core_ids=list(range(B)), **run_kw)
    except Exception:
        # transient NRT device wedge: retry once
        res = bass_utils.run_bass_kernel_spmd(
            nc, in_maps, core_ids=list(range(B)), **run_kw)
    out = np.empty((B, N, D), np.float32)
    attn_ = np.stack([np.asarray(res.results[b]["attn"]) for b in range(B)])
    for b in range(B):
        m, keep, nk = metas[b]
        xb = x[b]
        vbar = xb.mean(0) @ wv
        ybar = vbar @ w_out + b_out
        if nk <= kp:
            outc = np.asarray(res.results[b]["outc"])[:nk]
            out[b][keep] = outc + b_out
        else:
            out[b][keep] = _host_heads_fallback(xb, m, wq, wk, wv,
                                                w_out, b_out)
        out[b][~m] = ybar
    _CACHED["last_results"] = res
    return out, attn_
